# revision 28
# baseline (speedup 1.0000x reference)
"""TRN2 Bass kernel for nn_AVNNType1Linear.

Reference computation (B=2048, D_in=D_out=4096):
    act, carry = x[..., 0], x[..., 1]                  # x: [B, D_in, 2] f32
    act_out    = relu(act @ W.T + b)                   # [B, D_out]
    val        = 0.5*(mean(act, -1) + mean(carry, -1)) # [B]
    out        = stack([act_out, broadcast(val)], -1)  # [B, D_out, 2]

Distribution over 8 NeuronCores: 4-way data-parallel over batch x 2-way
tensor-parallel over output features. Per-core HBM traffic (f16 operands):
xaT 4.2MB + xn 8.4MB + W.T-half 16.8MB + out-f32 8.4MB ~= 38MB/core.

Per-core kernel: single-pass float16 matmul (full PE rate; fp32 PSUM
accumulate; measured scale-rel absmax error ~3.3e-4 vs the fp32
reference). The activator arrives host-transposed ([D_in, B_loc]) so the
contraction sits on the partition dim with clean DMAs. The GEMM runs in
o-tile pairs with all 8 PSUM banks live so every stationary act tile
feeds 2 consecutive matmuls. Bias is folded into PSUM via a K=1
ones-row matmul. Row sums (for the carry channel's broadcast mean) are
a DVE free-dim reduce over the naturally-laid-out interleaved x rows,
DMA'd mid-k-loop so they never front-run the w stream. The [b, (o,ch)]
interleaved output tile is assembled in SBUF so stores write contiguous
4KB rows, alternating between the Sync and GpSimd DMA queues.
"""

import os

import numpy as np

import concourse.mybir as mybir
import concourse.tile as tile
from concourse import bacc
from concourse.bass_utils import run_bass_kernel_spmd

B, D = 2048, 4096          # batch, D_in == D_out
M_SHARDS, F_SHARDS = 4, 2  # batch x feature grid over 8 cores
B_LOC = B // M_SHARDS      # 512 batch rows per core
O_LOC = D // F_SHARDS      # 2048 output features per core
KT = D // 128              # 32 contraction tiles
OT = O_LOC // 512          # 4 output tiles of 512
JT = B_LOC // 128          # 4 batch tiles of 128
KG = 8                     # activator SBUF tiles (groups of KT//KG k-tiles)
KPG = KT // KG

MM_DTYPE = os.environ.get("MM_DTYPE", "float16")


def _build():
    dt = mybir.dt
    mmdt = getattr(dt, MM_DTYPE)
    nc = bacc.Bacc("TRN2", target_bir_lowering=False, debug=False)
    xaT = nc.dram_tensor("xaT", [D, B_LOC], mmdt, kind="ExternalInput").ap()
    xn = nc.dram_tensor("xn", [B_LOC, 2 * D], mmdt, kind="ExternalInput").ap()
    wT = nc.dram_tensor("wT", [D, O_LOC], mmdt, kind="ExternalInput").ap()
    bias = nc.dram_tensor("bias", [1, O_LOC], mmdt, kind="ExternalInput").ap()
    ones = nc.dram_tensor("ones", [128, 128], mmdt, kind="ExternalInput").ap()
    out = nc.dram_tensor(
        "out", [B_LOC, O_LOC, 2], dt.float32, kind="ExternalOutput"
    ).ap()

    with tile.TileContext(nc) as tc:
        with (
            tc.tile_pool(name="persist", bufs=1) as persist,
            tc.tile_pool(name="wstream", bufs=6) as wpool,
            tc.tile_pool(name="xcpool", bufs=2) as xcpool,
            tc.tile_pool(name="opool", bufs=8) as opool,
            tc.tile_pool(name="small", bufs=1) as small,
            tc.tile_pool(name="ps", bufs=8, space="PSUM") as pspool,
        ):
            # --- persistent tiles; ones loads first (tiny, feeds PE
            # warmup), bias is deferred off the startup critical path ----
            ones_sb = persist.tile([128, 128], mmdt)
            nc.sync.dma_start(out=ones_sb, in_=ones)
            bias_sb = persist.tile([1, O_LOC], mmdt)

            # whole activator shard, [i%128, kt, b] layout, in KG chunks.
            # Only group 0 loads up front; later groups are emitted inside
            # the first k-loop so the w stream isn't queued behind them.
            act_g = [
                persist.tile([128, KPG, B_LOC], mmdt, tag=f"act{g}", name=f"act{g}")
                for g in range(KG)
            ]

            def load_act(g):
                nc.sync.dma_start(
                    out=act_g[g],
                    in_=xaT[g * KPG * 128 : (g + 1) * KPG * 128, :].rearrange(
                        "(kt p) b -> p kt b", p=128
                    ),
                )

            load_act(0)

            def act_tile(k, j):
                return act_g[k // KPG][:, k % KPG, j * 128 : (j + 1) * 128]

            # per-row total sums (both channels) -> val; the xn DMAs are
            # spread through the first k-loop so they share bandwidth with
            # the w stream instead of ever queuing ahead of it
            csum_sb = small.tile([128, JT], dt.float32)
            val_sb = small.tile([128, JT], dt.float32)

            def row_sums(j):
                xn_t = xcpool.tile([128, 2 * D], mmdt, tag="xn", name=f"xn_{j}")
                nc.sync.dma_start(out=xn_t, in_=xn[j * 128 : (j + 1) * 128, :])
                nc.vector.reduce_sum(
                    csum_sb[:, j : j + 1], xn_t, axis=mybir.AxisListType.X
                )

            # o-pairs: all 8 PSUM banks live so each stationary act tile
            # feeds 2 consecutive matmuls (the weight-load cost amortizes)
            STAG = 12  # pair-1 k-chunks before the j-major staggered tail

            def epilogue(ps_t, j, o, sync_store):
                o_sl = slice(o * 512, (o + 1) * 512)
                out_t = opool.tile(
                    [128, 512, 2], dt.float32, tag="out", name=f"out_{o}_{j}"
                )
                nc.vector.tensor_scalar_max(out_t[:, :, 0], ps_t, 0.0)
                nc.vector.tensor_scalar(
                    out_t[:, :, 1], ps_t, 0.0, val_sb[:, j : j + 1],
                    op0=mybir.AluOpType.mult, op1=mybir.AluOpType.add,
                )
                eng = nc.sync if sync_store else nc.gpsimd
                eng.dma_start(out=out[j * 128 : (j + 1) * 128, o_sl, :], in_=out_t)

            for op_ in range(OT // 2):
                o0 = 2 * op_
                pair_sl = slice(o0 * 512, (o0 + 2) * 512)
                last = op_ == OT // 2 - 1
                ps = [
                    [
                        pspool.tile(
                            [128, 512], dt.float32, tag="ps", name=f"ps_{o0}_{ot}_{j}"
                        )
                        for j in range(JT)
                    ]
                    for ot in range(2)
                ]
                if op_ == 0:
                    # PE warmup during the startup DMA wait: matmuls on the
                    # (tiny, early-loaded) ones tile flip the HAM clock gate
                    # to 8/8 before the real data lands; the garbage lands in
                    # ps[0][0] / ps[1][0], which the first real start=True
                    # matmuls clear anyway.
                    for i in range(40):
                        nc.tensor.matmul(
                            ps[i % 2][0][:, 0:128],
                            ones_sb,
                            ones_sb,
                            start=True, stop=True,
                            skip_group_check=True,
                        )
                n_kp = KT // 2 if not last else STAG
                w_tiles = {}
                for kp in range(KT // 2):
                    if op_ == 0 and kp == 1:
                        nc.sync.dma_start(out=bias_sb, in_=bias)
                    if op_ == 0 and 1 <= kp <= KG - 1:
                        load_act(kp)
                    if op_ == 0 and kp in (9, 11, 13, 15):
                        row_sums((kp - 9) // 2)
                    w_t = wpool.tile(
                        [128, 2, 2, 512], mmdt, tag="wt", name=f"wt_{o0}_{kp}"
                    )
                    nc.sync.dma_start(
                        out=w_t,
                        in_=wT[kp * 256 : (kp + 1) * 256, pair_sl].rearrange(
                            "(kt p) (ot n) -> p kt ot n", p=128, n=512
                        ),
                    )
                    w_tiles[kp] = w_t
                    if kp >= n_kp:
                        continue  # tail k-chunks of the last pair run j-major
                    # two k-tiles per w fetch, two o-tiles per stationary
                    for kk in range(2):
                        k = 2 * kp + kk
                        for j in range(JT):
                            for ot in range(2):
                                nc.tensor.matmul(
                                    ps[ot][j], act_tile(k, j), w_t[:, kk, ot, :],
                                    start=(k == 0), stop=(k == KT - 1),
                                )
                        if k == 16:
                            # bias: ones-row (K=1) x bias-row accumulate;
                            # grouped so the ones stationary loads once
                            for j in range(JT):
                                for ot in range(2):
                                    nc.tensor.matmul(
                                        ps[ot][j], ones_sb[0:1, :],
                                        bias_sb[
                                            0:1,
                                            (o0 + ot) * 512 : (o0 + ot + 1) * 512,
                                        ],
                                        start=False, stop=False,
                                    )
                if op_ == 0:
                    # val = total row sum / (2*D)
                    nc.vector.tensor_scalar_mul(val_sb, csum_sb, 1.0 / (2 * D))
                    for j in range(JT):
                        for ot in range(2):
                            epilogue(ps[ot][j], j, o0 + ot, (j + ot) % 2 == 0)
                else:
                    # staggered j-major tail: each batch tile finishes its
                    # remaining k-chunks and streams its epilogue + store out
                    # while the later batch tiles still compute
                    for j in range(JT):
                        for kp in range(STAG, KT // 2):
                            for kk in range(2):
                                k = 2 * kp + kk
                                for ot in range(2):
                                    nc.tensor.matmul(
                                        ps[ot][j], act_tile(k, j),
                                        w_tiles[kp][:, kk, ot, :],
                                        start=(k == 0), stop=(k == KT - 1),
                                    )
                        for ot in range(2):
                            epilogue(ps[ot][j], j, o0 + ot, True)
    nc.compile()
    return nc


def _np_mmdt():
    if MM_DTYPE == "float16":
        return np.float16
    if MM_DTYPE == "bfloat16":
        import ml_dtypes

        return np.dtype(ml_dtypes.bfloat16)
    return np.float32  # float32 / float32r


def _shard_inputs(x, W, b):
    ndt = _np_mmdt()
    x = np.ascontiguousarray(x, dtype=np.float32)
    W = np.asarray(W, dtype=np.float32)
    b = np.asarray(b, dtype=np.float32)
    wT_shards = [
        np.ascontiguousarray(W[c * O_LOC : (c + 1) * O_LOC, :].T).astype(ndt)
        for c in range(F_SHARDS)
    ]
    bias_shards = [
        b[c * O_LOC : (c + 1) * O_LOC].reshape(1, O_LOC).astype(ndt)
        for c in range(F_SHARDS)
    ]
    ones = np.ones((128, 128), dtype=ndt)
    in_maps = []
    for core in range(M_SHARDS * F_SHARDS):
        r, c = core % M_SHARDS, core // M_SHARDS
        b_sl = slice(r * B_LOC, (r + 1) * B_LOC)
        in_maps.append(
            dict(
                xaT=np.ascontiguousarray(x[b_sl, :, 0].T).astype(ndt),
                xn=x[b_sl].reshape(B_LOC, 2 * D).astype(ndt),
                wT=wT_shards[c],
                bias=bias_shards[c],
                ones=ones,
            )
        )
    return in_maps


def _gather(results):
    out = np.empty((B, D, 2), dtype=np.float32)
    for core, r in enumerate(results):
        m, c = core % M_SHARDS, core // M_SHARDS
        out[m * B_LOC : (m + 1) * B_LOC, c * O_LOC : (c + 1) * O_LOC, :] = r["out"]
    return out


def _run(x, W, b, trace=False, **spmd_kwargs):
    in_maps = _shard_inputs(x, W, b)
    nc = _build()
    res = run_bass_kernel_spmd(
        nc, in_maps, core_ids=list(range(8)), trace=trace, **spmd_kwargs
    )
    return _gather(res.results), res


def kernel(x, W, b):
    out, _ = _run(x, W, b, trace=False)
    return out
